# revision 1
# baseline (speedup 1.0000x reference)
"""Trainium2 Bass kernel for nn_AttentionalAggregator (GAT-style aggregation).

Computation (per (b, h) node):
    xw_k    = x_k @ W                 (k = self + 25 neighbours)
    s_self  = x_self . (W @ a_self)
    t_k     = x_k . (W @ a_neigh)
    u_k     = leaky_relu(s_self + t_k, 0.2)
    attn    = softmax_k(u_k)
    out     = relu(sum_k attn_k * x_k @ W) = relu((sum_k attn_k x_k) @ W)

Distribution: data-parallel over the batch axis, 128 batches per core x 8 cores.

Per-core device pipeline (32 tiles of 128 (b,h)-rows):
  - scores:   PE matmuls, stationary = transposed x block [f, 128 rows] (bf16),
              moving = [wa_self_hi, wa_neigh_hi, wa_self_lo, wa_neigh_lo]
  - softmax:  DVE/ACT small ops on [128, 26]
  - weighted sum over k: PE matmuls, stationary = x rows [104=(4 bh x 26 k), 128 f],
              moving = block-diag attention [104, 4] -> acc^T [128 f, 128 bh] in PSUM
  - final:    acc^T (fp32) as stationary vs W (fp32) -> h [128 bh, 128 d], relu on
              the ScalarE PSUM->SBUF evacuation.

The host stages x in the two layouts the PE needs (feature-major for the score
pass, row-major for the weighted sum), both in bf16, so HBM traffic matches the
fp32 single-copy roofline. All matmul accumulation is fp32 (PSUM); the final
acc @ W matmul runs in full fp32.
"""

import sys

sys.path.insert(0, "/opt/trn_rl_repo")

from contextlib import ExitStack

import ml_dtypes
import numpy as np

import concourse.bass as bass  # noqa: F401  (import keeps bass registered)
import concourse.tile as tile
from concourse import bacc, mybir
from concourse.bass_interp import get_hw_module
from concourse.bass_utils import run_bass_kernel_spmd
from concourse.masks import make_identity

BF16 = mybir.dt.bfloat16
F32 = mybir.dt.float32
BF16_NP = ml_dtypes.bfloat16

B, H, NNEIGH, F, D = 1024, 32, 25, 128, 128
K = NNEIGH + 1  # 26 (self + neighbours)
NCORES = 8
BSH = B // NCORES  # 128 batches per core
BH = BSH * H  # 4096 rows per core
TILES = BH // 128  # 32
GROUPS = 32  # groups of 4 rows per tile
KPAD = 32  # k-block padded to 32 (partition bases must be 32-aligned)
KP = 4 * KPAD  # 128 partitions for the weighted-sum stationary

NEG_SLOPE = 0.2

_CACHE = {}


def build_module(n_tiles=TILES, variant=False, ramp_n=4, ramp_ext=False):
    nc = bacc.Bacc(
        "TRN2",
        target_bir_lowering=False,
        debug=False,
        num_devices=NCORES,
    )
    xT = nc.dram_tensor("xT", [n_tiles, 128, K * 128], BF16, kind="ExternalInput").ap()
    xN = nc.dram_tensor(
        "xN", [n_tiles, KP, GROUPS * 128], BF16, kind="ExternalInput"
    ).ap()
    wa4 = nc.dram_tensor("wa4", [128, 4], BF16, kind="ExternalInput").ap()
    wmat = nc.dram_tensor("wmat", [128, 128], F32, kind="ExternalInput").ap()
    out = nc.dram_tensor("out", [n_tiles, 128, 128], F32, kind="ExternalOutput").ap()

    add = mybir.AluOpType.add
    mult = mybir.AluOpType.mult
    vmax = mybir.AluOpType.max

    with tile.TileContext(nc) as tc, ExitStack() as ctx:
        xT_pool = ctx.enter_context(tc.tile_pool(name="xT", bufs=8))
        xN_pool = ctx.enter_context(tc.tile_pool(name="xN", bufs=8))
        const_pool = ctx.enter_context(tc.tile_pool(name="const", bufs=1))
        sm_pool = ctx.enter_context(tc.tile_pool(name="sm", bufs=4))
        ab_pool = ctx.enter_context(tc.tile_pool(name="ab", bufs=4))
        acc_pool = ctx.enter_context(tc.tile_pool(name="accT", bufs=3))
        out_pool = ctx.enter_context(tc.tile_pool(name="outsb", bufs=4))
        ps_s = ctx.enter_context(tc.tile_pool(name="ps_s", bufs=2, space="PSUM"))
        ps_at = ctx.enter_context(tc.tile_pool(name="ps_at", bufs=2, space="PSUM"))
        ps_acc = ctx.enter_context(tc.tile_pool(name="ps_acc", bufs=2, space="PSUM"))
        ps_h = ctx.enter_context(tc.tile_pool(name="ps_h", bufs=2, space="PSUM"))

        wa_sb = const_pool.tile([128, 4], BF16)
        nc.sync.dma_start(wa_sb[:], wa4[:])
        w_sb = const_pool.tile([128, 128], F32)
        nc.sync.dma_start(w_sb[:], wmat[:])
        ident = const_pool.tile([128, 128], BF16)
        make_identity(nc, ident[:])

        for t in range(n_tiles):
            xT_t = xT_pool.tile([128, K * 128], BF16)
            nc.sync.dma_start(xT_t[:], xT[t])
            xN_t = xN_pool.tile([KP, GROUPS * 128], BF16)
            nc.scalar.dma_start(xN_t[:], xN[t])

            # scores: out[j, c] = sum_f x[row j of k-block, f] * wa4[f, c]
            s_ps = ps_s.tile([128, 4 * K], F32)
            for k in range(K):
                nc.tensor.matmul(
                    s_ps[:, 4 * k : 4 * k + 4],
                    lhsT=xT_t[:, 128 * k : 128 * (k + 1)],
                    rhs=wa_sb[:],
                    start=True,
                    stop=True,
                )

            # softmax over k (26)
            s_sb = sm_pool.tile([128, 4 * K], F32, tag="s_sb")
            # first tiles: keep ACT's instruction stream free of compute waits
            # so its DMA-trigger run-ahead fills the xN ring during ramp-up
            if t < ramp_n:
                nc.vector.tensor_copy(s_sb[:], s_ps[:])
            else:
                nc.scalar.copy(s_sb[:], s_ps[:])
            t_sc = sm_pool.tile([128, K], F32, tag="t_sc")
            nc.vector.tensor_tensor(
                t_sc[:], s_sb[:, 1 : 4 * K : 4], s_sb[:, 3 : 4 * K : 4], op=add
            )
            s_a = sm_pool.tile([128, 1], F32, tag="s_a")
            nc.vector.tensor_tensor(s_a[:], s_sb[:, 0:1], s_sb[:, 2:3], op=add)
            u = sm_pool.tile([128, K], F32, tag="u")
            nc.vector.tensor_scalar(u[:], t_sc[:], s_a[:], None, op0=add)
            u2 = sm_pool.tile([128, K], F32, tag="u2")
            nc.vector.tensor_scalar(u2[:], u[:], NEG_SLOPE, None, op0=mult)
            nc.vector.tensor_tensor(u[:], u[:], u2[:], op=vmax)
            e = sm_pool.tile([128, K], F32, tag="e")
            den = sm_pool.tile([128, 1], F32, tag="den")
            if variant:
                nc.scalar.activation(e[:], u[:], mybir.ActivationFunctionType.Exp)
                nc.vector.tensor_reduce(
                    den[:], e[:], axis=mybir.AxisListType.X, op=add
                )
            else:
                nc.scalar.activation(
                    e[:], u[:], mybir.ActivationFunctionType.Exp, accum_out=den[:]
                )
            rec = sm_pool.tile([128, 1], F32, tag="rec")
            nc.vector.reciprocal(rec[:], den[:])
            attn = sm_pool.tile([128, 32], BF16, tag="attn")
            nc.vector.memset(attn[:, K:32], 0.0)
            nc.vector.tensor_scalar(attn[:, 0:K], e[:], rec[:], None, op0=mult)

            # transpose attention to [k, row] and scatter to block-diag [104, 128]
            at_ps = ps_at.tile([32, 128], BF16)
            nc.tensor.transpose(at_ps[:], attn[:], ident[:])
            at_sb = sm_pool.tile([32, 128], BF16, tag="at_sb")
            nc.vector.tensor_copy(at_sb[:], at_ps[:])
            ab = ab_pool.tile([KP, 128], BF16)
            nc.vector.memset(ab[:], 0.0)
            for q in range(4):
                nc.vector.tensor_copy(
                    ab[KPAD * q : KPAD * q + K, q : 128 : 4], at_sb[0:K, q : 128 : 4]
                )

            # weighted sum over k -> acc^T [128 f, 128 rows]
            acc_ps = ps_acc.tile([128, 128], F32)
            for g in range(GROUPS):
                nc.tensor.matmul(
                    acc_ps[:, 4 * g : 4 * g + 4],
                    lhsT=xN_t[:, 128 * g : 128 * (g + 1)],
                    rhs=ab[:, 4 * g : 4 * g + 4],
                    start=True,
                    stop=True,
                )
            accT = acc_pool.tile([128, 128], F32)
            if ramp_ext and t < 4:
                nc.vector.tensor_copy(accT[:], acc_ps[:])
            else:
                nc.scalar.copy(accT[:], acc_ps[:])

            # h = acc @ W (full fp32), relu on evacuation
            h_ps = ps_h.tile([128, 128], F32)
            nc.tensor.matmul(h_ps[:], lhsT=accT[:], rhs=w_sb[:], start=True, stop=True)
            o_sb = out_pool.tile([128, 128], F32)
            if ramp_ext and t < 4:
                nc.vector.tensor_scalar(o_sb[:], h_ps[:], 0.0, None, op0=vmax)
            else:
                nc.scalar.activation(
                    o_sb[:], h_ps[:], mybir.ActivationFunctionType.Relu
                )
            nc.sync.dma_start(out[t], o_sb[:])

    nc.compile()
    nc.m = get_hw_module(nc.m)
    return nc


def _split_hi_lo(v):
    hi = v.astype(BF16_NP)
    lo = (v - hi.astype(np.float32)).astype(BF16_NP)
    return hi, lo


def stage_inputs(x_self, x_neigh, w_feat, a_self, a_neigh, n_tiles=TILES):
    """Build the per-core input maps (host-side layout staging)."""
    x_self = np.asarray(x_self, np.float32)
    x_neigh = np.asarray(x_neigh, np.float32)
    w_feat = np.asarray(w_feat, np.float32)
    a_self = np.asarray(a_self, np.float32)
    a_neigh = np.asarray(a_neigh, np.float32)

    wa_s = (w_feat @ a_self)[:, 0]
    wa_n = (w_feat @ a_neigh)[:, 0]
    was_h, was_l = _split_hi_lo(wa_s)
    wan_h, wan_l = _split_hi_lo(wa_n)
    wa4 = np.ascontiguousarray(np.stack([was_h, wan_h, was_l, wan_l], axis=1))

    x_all = np.concatenate([x_self[:, :, None, :], x_neigh], axis=2)  # [B,H,26,F]

    in_maps = []
    for c in range(NCORES):
        xa = x_all[c * BSH : (c + 1) * BSH].reshape(BH, K, F)
        xa_b = xa.astype(BF16_NP)
        rows = n_tiles * 128
        # xT[t, f, k*128 + j] = x[128 t + j, k, f]
        xT = np.ascontiguousarray(
            xa_b[:rows].reshape(n_tiles, 128, K, F).transpose(0, 3, 2, 1)
        ).reshape(n_tiles, F, K * 128)
        # xN[t, 32 q + k, 128 g + f] = x[128 t + 4 g + q, k, f]  (k >= 26 zero pad)
        xNu = np.zeros((n_tiles, 4, KPAD, GROUPS, F), dtype=BF16_NP)
        xNu[:, :, :K] = (
            xa_b[:rows].reshape(n_tiles, GROUPS, 4, K, F).transpose(0, 2, 3, 1, 4)
        )
        xN = np.ascontiguousarray(xNu).reshape(n_tiles, KP, GROUPS * F)
        in_maps.append({"xT": xT, "xN": xN, "wa4": wa4, "wmat": w_feat})
    return in_maps


def _install_ntff_shim():
    """Provide antenv.axon_hooks (missing in this image) so trace=True works."""
    import types

    if "antenv.axon_hooks" in sys.modules:
        return
    mod = types.ModuleType("antenv.axon_hooks")
    holder = [None]
    mod.get_axon_ntff_profile_hook = lambda: holder[0]
    mod.set_axon_ntff_profile_hook = lambda h: holder.__setitem__(0, h)
    sys.modules["antenv.axon_hooks"] = mod
    try:
        import antenv

        antenv.axon_hooks = mod
    except ImportError:
        pass
    try:
        from trn_agent_boot.trn_boot import _ntff_profile_via_ctypes

        hook = _ntff_profile_via_ctypes("/opt/axon/libaxon_pjrt.so")
        if hook is not None:
            mod.set_axon_ntff_profile_hook(hook)
    except Exception as e:  # pragma: no cover
        print("ntff shim: hook install failed:", e)


def run(inputs, trace=False, trace_cores=None):
    """Run on the 8 NeuronCores; returns (output, BassKernelResults)."""
    if trace:
        _install_ntff_shim()
    if "nc" not in _CACHE:
        _CACHE["nc"] = build_module()
    nc = _CACHE["nc"]
    in_maps = stage_inputs(**inputs)
    kwargs = {}
    if trace:
        kwargs["trace"] = True
        if trace_cores is not None:
            kwargs["trace_cores"] = trace_cores
    res = run_bass_kernel_spmd(nc, in_maps, core_ids=list(range(NCORES)), **kwargs)
    outs = [res.results[c]["out"].reshape(BSH, H, D) for c in range(NCORES)]
    return np.concatenate(outs, axis=0), res


def kernel(**inputs):
    out, _ = run(inputs, trace=False)
    return out

